# revision 26
# baseline (speedup 1.0000x reference)
"""Trainium2 Bass kernel for nn_CMAModel (memory-augmented causal attention).

Sharding: 8 cores = 2 batches x 4 head-groups. Each core handles one batch and
4 heads (256 channels); the output projection is row-parallel and the 4
per-batch partials are summed on the host.

v2: chunk-major software pipeline so the tensor engine always has independent
matmul work queued (keeps the HAM clock gate at 2.4 GHz):
  projections for all 512-col T-chunks are issued first (separate PSUM pool
  tags: proj/outproj=2, scores=2, accum=4 banks), then per chunk: attention
  for the 4 heads, depthwise conv, out-projection. The Tile scheduler
  interleaves projection and out-projection matmuls into exp-wait bubbles.
Combine avoids the v1 DRAM round trips: 1/Z via reciprocal_approx_fast on the
PV ones-row, one partition-broadcast DMA per head-chunk, gate-row broadcasts
precomputed per chunk off the critical path.
"""
import contextlib
import ctypes
import os
import sys
import types

import numpy as np

# ---------------------------------------------------------------- constants
B, T, C = 2, 2048, 1024
H, HD = 16, 64
M = 256
G = 4                 # head-groups (cores per batch)
HPG = H // G          # 4 heads per core
CPG = HPG * HD        # 256 channels per core
S = T + 2 * M         # 2560 kv rows
SM = 2 * M            # 512 memory rows
NKT = C // 128        # 8 contraction tiles
NST = S // 128        # 20 S tiles (16 chunk + 4 mem)
TC = 512              # T chunk size
NTC = T // TC         # 4
K_CONV = 4
SCALE = 1.0 / float(np.sqrt(HD))

_MM_DTYPE = os.environ.get("BASS_MM_DTYPE", "float32r")
_ZBCAST_SBUF = bool(int(os.environ.get("BASS_ZBCAST_SBUF", "0")))
_RECIP_EXACT = bool(int(os.environ.get("BASS_RECIP_EXACT", "0")))

_BUILT = None


# ------------------------------------------------------- axon NTFF hook shim
def _install_ntff_hook():
    """The agent image lacks antenv.axon_hooks; synthesize it so
    run_bass_kernel_spmd(trace=True) can capture NTFF profiles."""
    if "antenv.axon_hooks" in sys.modules:
        return
    so_path = "/opt/axon/libaxon_pjrt.so"
    hook = None
    if os.path.exists(so_path):
        try:
            lib = ctypes.CDLL(so_path)
            if hasattr(lib, "axon_start_nrt_profile"):
                lib.axon_start_nrt_profile.argtypes = [
                    ctypes.POINTER(ctypes.c_int64),
                    ctypes.c_size_t,
                ]
                lib.axon_start_nrt_profile.restype = ctypes.c_int64
                lib.axon_stop_nrt_profile.argtypes = [ctypes.c_char_p]
                lib.axon_stop_nrt_profile.restype = ctypes.c_int64

                @contextlib.contextmanager
                def _hook(output_dir, device_ids):
                    import jax

                    jax.devices()
                    if device_ids:
                        ids = (ctypes.c_int64 * len(device_ids))(*device_ids)
                        rc = lib.axon_start_nrt_profile(ids, len(device_ids))
                    else:
                        rc = lib.axon_start_nrt_profile(None, 0)
                    if rc != 0:
                        raise RuntimeError(f"axon_start_nrt_profile rc={rc}")
                    try:
                        yield
                    finally:
                        n = lib.axon_stop_nrt_profile(str(output_dir).encode())
                        if n < 0:
                            raise RuntimeError(f"axon_stop_nrt_profile rc={n}")

                hook = _hook
        except OSError:
            pass
    mod = types.ModuleType("antenv.axon_hooks")
    mod.get_axon_ntff_profile_hook = lambda: hook
    mod.set_axon_ntff_profile_hook = lambda h: None
    sys.modules["antenv.axon_hooks"] = mod


# ------------------------------------------------------------- device build
def _build_program():
    import concourse.tile as tile
    from concourse import bacc, mybir
    from concourse.masks import make_upper_triangular

    f32 = mybir.dt.float32
    bf16 = mybir.dt.bfloat16
    mdt = getattr(mybir.dt, _MM_DTYPE)  # dtype of all matmul operands

    nc = bacc.Bacc("TRN2", target_bir_lowering=False, debug=False, num_devices=8)

    xT = nc.dram_tensor("xT", [C, T], mdt, kind="ExternalInput").ap()
    memT = nc.dram_tensor("memT", [C, SM], mdt, kind="ExternalInput").ap()
    WqT = nc.dram_tensor("WqT", [C, CPG], mdt, kind="ExternalInput").ap()
    WkT = nc.dram_tensor("WkT", [C, CPG], mdt, kind="ExternalInput").ap()
    WvTa = nc.dram_tensor("WvTa", [C, 65 * HPG], mdt, kind="ExternalInput").ap()
    WgT = nc.dram_tensor("WgT", [C, HPG], mdt, kind="ExternalInput").ap()
    gbn = nc.dram_tensor("gbn", [HPG, 1], f32, kind="ExternalInput").ap()
    WoT = nc.dram_tensor("WoT", [CPG, C], mdt, kind="ExternalInput").ap()
    cw = nc.dram_tensor("cw", [CPG, K_CONV], f32, kind="ExternalInput").ap()
    cb = nc.dram_tensor("cb", [CPG, 1], f32, kind="ExternalInput").ap()
    out = nc.dram_tensor("out", [T, C], f32, kind="ExternalOutput").ap()
    dbg_on = bool(int(os.environ.get("BASS_DBG", "0")))
    dbgZ = (
        nc.dram_tensor("dbgZ", [NTC * HPG, TC], f32, kind="ExternalOutput").ap()
        if dbg_on else None
    )
    dbgY = (
        nc.dram_tensor("dbgY", [128, 2 * T], mybir.dt.bfloat16,
                       kind="ExternalOutput").ap()
        if dbg_on else None
    )

    Exp = mybir.ActivationFunctionType.Exp

    with tile.TileContext(nc) as tc:
        with contextlib.ExitStack() as ctx:
            const = ctx.enter_context(tc.tile_pool(name="const", bufs=1))
            xpool = ctx.enter_context(tc.tile_pool(name="xpool", bufs=2))
            sb = ctx.enter_context(tc.tile_pool(name="sb", bufs=1))
            work = ctx.enter_context(tc.tile_pool(name="work", bufs=3))
            small = ctx.enter_context(tc.tile_pool(name="small", bufs=1))
            pps = ctx.enter_context(tc.tile_pool(name="pps", bufs=1, space="PSUM"))
            psc = ctx.enter_context(tc.tile_pool(name="psc", bufs=2, space="PSUM"))
            ppa = ctx.enter_context(tc.tile_pool(name="ppa", bufs=3, space="PSUM"))
            drs = ctx.enter_context(tc.tile_pool(name="drs", bufs=4, space="DRAM"))

            # ---- constants / weights (q/k first so chunk-0 proj starts early)
            wq_s = const.tile([128, NKT, CPG], mdt)
            nc.sync.dma_start(out=wq_s, in_=WqT.rearrange("(a p) n -> p a n", p=128))
            wk_s = const.tile([128, NKT, CPG], mdt)
            nc.sync.dma_start(out=wk_s, in_=WkT.rearrange("(a p) n -> p a n", p=128))
            wva_s = const.tile([128, NKT, 65 * HPG], mdt)
            nc.sync.dma_start(out=wva_s, in_=WvTa.rearrange("(a p) n -> p a n", p=128))
            wg_s = const.tile([128, NKT, HPG], mdt)
            nc.sync.dma_start(out=wg_s, in_=WgT.rearrange("(a p) n -> p a n", p=128))
            cw_s = const.tile([128, 2, K_CONV], f32)
            nc.sync.dma_start(out=cw_s, in_=cw.rearrange("(a p) n -> p a n", p=128))
            cb_s = const.tile([128, 2, 1], f32)
            nc.sync.dma_start(out=cb_s, in_=cb.rearrange("(a p) n -> p a n", p=128))
            gbn_s = const.tile([HPG, 1], f32)
            nc.sync.dma_start(out=gbn_s, in_=gbn)
            wo_s = const.tile([128, 2, C], mdt)
            nc.sync.dma_start(out=wo_s, in_=WoT.rearrange("(a p) n -> p a n", p=128))

            tri = const.tile([128, 128], f32)
            make_upper_triangular(nc, tri, val=1.0, diag=True)

            # ---- persistent activations
            qT_s = sb.tile([128, 2, T], mdt)
            kT_s = sb.tile([128, 2, S], mdt)
            V_s = sb.tile([128, NST, 65 * HPG], mdt)  # [128, 20, 260]
            Y_s = sb.tile([128, 2, T], bf16)          # combined attention output

            gd = drs.tile([NTC, HPG, TC], bf16, tag="gd", bufs=1)  # gate rows

            xTr = xT.rearrange("(a p) t -> p a t", p=128)

            def ones_fill(st):
                oc = V_s[:, st, 64:65 * HPG:65]
                nc.vector.tensor_scalar(
                    oc, oc, 0.0, 1.0,
                    mybir.AluOpType.mult, mybir.AluOpType.add,
                )

            xh_dmas = {}  # j -> list of x-load DMA instructions (for rationing)

            def proj_chunk(j):
                """q/k/V/gate projections for T columns [j*TC, (j+1)*TC)."""
                t0 = j * TC
                xh = xpool.tile([128, NKT, TC], mdt, tag="xc", name=f"xh{j}")
                xh_dmas[j] = []
                for k in range(NKT):
                    xh_dmas[j].append(
                        nc.sync.dma_start(out=xh[:, k, :], in_=xTr[:, k, t0:t0 + TC])
                    )
                for m in range(2):
                    pq = pps.tile([128, TC], f32, tag="ps")
                    for k in range(NKT):
                        nc.tensor.matmul(
                            pq,
                            wq_s[:, k, m * 128:(m + 1) * 128],
                            xh[:, k, :],
                            start=(k == 0),
                            stop=(k == NKT - 1),
                        )
                    nc.vector.tensor_copy(qT_s[:, m, t0:t0 + TC], pq)
                    pk = pps.tile([128, TC], f32, tag="ps")
                    for k in range(NKT):
                        nc.tensor.matmul(
                            pk,
                            wk_s[:, k, m * 128:(m + 1) * 128],
                            xh[:, k, :],
                            start=(k == 0),
                            stop=(k == NKT - 1),
                        )
                    nc.vector.tensor_copy(kT_s[:, m, t0:t0 + TC], pk)
                for mt in range(TC // 128):
                    st = j * 4 + mt
                    pv = pps.tile([128, 65 * HPG], f32, tag="ps")
                    for k in range(NKT):
                        nc.tensor.matmul(
                            pv,
                            xh[:, k, mt * 128:(mt + 1) * 128],
                            wva_s[:, k, :],
                            start=(k == 0),
                            stop=(k == NKT - 1),
                        )
                    nc.vector.tensor_copy(V_s[:, st, :], pv)
                    ones_fill(st)
                # gate: logits -> exp(-(l+b)) -> +1 -> 1/x -> DRAM -> bcast
                pg = pps.tile([HPG, TC], f32, tag="ps")
                for k in range(NKT):
                    nc.tensor.matmul(
                        pg,
                        wg_s[:, k, :],
                        xh[:, k, :],
                        start=(k == 0),
                        stop=(k == NKT - 1),
                    )
                gtmp = small.tile([HPG, TC], f32, tag="gtmp", bufs=2)
                nc.scalar.activation(gtmp, pg, Exp, bias=gbn_s, scale=-1.0)
                gr = small.tile([128, HPG, TC // 128], f32, tag="gr", bufs=2)
                for hl in range(HPG):
                    nc.sync.dma_start(out=gr[:, hl, :], in_=gtmp[hl:hl + 1, :])
                nc.vector.tensor_scalar_add(gr, gr, 1.0)
                nc.vector.reciprocal(gr, gr)
                grb = small.tile([128, HPG, TC // 128], bf16, tag="grb", bufs=2)
                nc.vector.tensor_copy(grb, gr)
                for hl in range(HPG):
                    nc.sync.dma_start(out=gd[j, hl:hl + 1, :], in_=grb[:, hl, :])
                gb = small.tile([64, HPG, TC], bf16, tag="gb", bufs=2,
                                name=f"gb{j}")
                for hl in range(HPG):
                    nc.gpsimd.dma_start(
                        out=gb[:, hl, :],
                        in_=gd[j, hl:hl + 1, :].partition_broadcast(64),
                    )
                return gb

            def proj_mem():
                mems = xpool.tile([128, NKT, SM], mdt, tag="xc", name="mems")
                nc.sync.dma_start(
                    out=mems, in_=memT.rearrange("(a p) t -> p a t", p=128)
                )
                for m in range(2):
                    pk = pps.tile([128, SM], f32, tag="ps")
                    for half in range(2):
                        for k in range(NKT):
                            nc.tensor.matmul(
                                pk[:, half * 256:(half + 1) * 256],
                                wk_s[:, k, m * 128:(m + 1) * 128],
                                mems[:, k, half * 256:(half + 1) * 256],
                                start=(k == 0),
                                stop=(k == NKT - 1),
                            )
                    nc.vector.tensor_copy(kT_s[:, m, T:], pk)
                for mt in range(SM // 128):
                    st = 16 + mt
                    pv = pps.tile([128, 65 * HPG], f32, tag="ps")
                    for k in range(NKT):
                        nc.tensor.matmul(
                            pv,
                            mems[:, k, mt * 128:(mt + 1) * 128],
                            wva_s[:, k, :],
                            start=(k == 0),
                            stop=(k == NKT - 1),
                        )
                    nc.vector.tensor_copy(V_s[:, st, :], pv)
                    ones_fill(st)

            def attn_head(j, hl, gb):
                """scores+exp+mask+PV+combine for head hl on chunk j."""
                mq, par = divmod(hl, 2)
                ro = 64 * par
                vc = 65 * hl
                nct = 4 * (j + 1)
                Ac = ppa.tile([128, TC], f32, tag="pa")
                Am = ppa.tile([128, TC], f32, tag="pa")
                first_mm = None
                for ip in range((nct + 4) // 2):
                    sc2 = psc.tile([128, 2, TC], f32, tag="sc")
                    Pt2 = work.tile([128, 2, TC], mdt, tag="P")
                    halves = []
                    for half in range(2):
                        i = 2 * ip + half
                        is_mem = i >= nct
                        si = (16 + i - nct) if is_mem else i
                        off = 0
                        if not is_mem and si >= 4 * j:
                            off = 128 * si - TC * j
                        halves.append((i, is_mem, si, off))
                        mm = nc.tensor.matmul(
                            sc2[:, half, off:],
                            kT_s[ro:ro + 64, mq, si * 128:(si + 1) * 128],
                            qT_s[ro:ro + 64, mq, TC * j + off:TC * (j + 1)],
                            start=True,
                            stop=True,
                        )
                        if first_mm is None:
                            first_mm = mm
                    if halves[1][3] > 0:
                        # diagonal pair: separate trimmed exps (avoid
                        # reading unwritten PSUM between the valid ranges)
                        for half, (i, is_mem, si, off) in enumerate(halves):
                            nc.scalar.activation(
                                Pt2[:, half, off:], sc2[:, half, off:],
                                Exp, scale=SCALE,
                            )
                    else:
                        off0 = halves[0][3]
                        nc.scalar.activation(
                            Pt2[:, :, :].rearrange("p a t -> p (a t)")[:, off0:],
                            sc2[:, :, :].rearrange("p a t -> p (a t)")[:, off0:],
                            Exp, scale=SCALE,
                        )
                    for half, (i, is_mem, si, off) in enumerate(halves):
                        if not is_mem and si >= 4 * j:
                            nc.vector.tensor_mul(
                                Pt2[:, half, off:off + 128],
                                Pt2[:, half, off:off + 128], tri
                            )
                        dst = Am if is_mem else Ac
                        first = (i == 0) or (is_mem and i == nct)
                        last = (i == nct - 1) or (i == nct + 3)
                        nc.tensor.matmul(
                            dst[0:65, off:],
                            V_s[:, si, vc:vc + 65],
                            Pt2[:, half, off:],
                            start=first,
                            stop=last,
                        )
                # ---- combine: Y = (Ac + g*Am) / Z
                zrow = small.tile([128, TC], f32, tag="zrow", bufs=3)
                zr = zrow[64:65, :]
                nc.vector.tensor_copy(zr, Ac[64:65, :])
                nc.vector.tensor_add(zr, zr, Am[64:65, :])
                zq = small.tile([128, TC // 128], f32, tag="zq", bufs=3)
                nc.sync.dma_start(out=zq, in_=zr)
                nc.vector.reciprocal(zq, zq)
                zb = small.tile([64, TC], f32, tag="zb", bufs=3)
                if dbgZ is not None:
                    idx = j * HPG + hl
                    nc.sync.dma_start(out=dbgZ[idx:idx + 1, :], in_=zq)
                    nc.sync.dma_start(
                        out=zb, in_=dbgZ[idx:idx + 1, :].partition_broadcast(64)
                    )
                else:
                    zd = drs.tile([1, TC], f32, tag="zd", bufs=4)
                    nc.sync.dma_start(out=zd, in_=zq)
                    nc.sync.dma_start(out=zb, in_=zd.partition_broadcast(64))
                t1 = small.tile([64, TC], f32, tag="t1", bufs=2)
                nc.vector.tensor_mul(t1, gb[:, hl, :], Am[0:64, :])
                nc.vector.tensor_add(t1, t1, Ac[0:64, :])
                if par == 0:
                    nc.vector.tensor_mul(
                        Y_s[0:64, mq, TC * j:TC * (j + 1)], t1, zb
                    )
                else:
                    yt = small.tile([64, TC], bf16, tag="yt", bufs=3)
                    nc.vector.tensor_mul(yt, t1, zb)
                    nc.gpsimd.dma_start(
                        out=Y_s[64:128, mq, TC * j:TC * (j + 1)], in_=yt
                    )
                return first_mm

            def conv_pair(j, p, R):
                """depthwise causal conv + residual + bias for chunk j, bank p."""
                t0 = j * TC
                y = Y_s[:, p, t0:t0 + TC]
                nc.vector.tensor_scalar_add(R[:, p, :], y, cb_s[:, p, :])
                for k in range(K_CONV):
                    sh = K_CONV - 1 - k
                    ctmp = small.tile([128, TC], f32, tag="ctmp", bufs=2)
                    if sh == 0:
                        nc.vector.tensor_scalar_mul(ctmp, y, cw_s[:, p, k:k + 1])
                        nc.vector.tensor_add(R[:, p, :], R[:, p, :], ctmp)
                    elif t0 - sh >= 0:
                        nc.vector.tensor_scalar_mul(
                            ctmp, Y_s[:, p, t0 - sh:t0 + TC - sh],
                            cw_s[:, p, k:k + 1],
                        )
                        nc.vector.tensor_add(R[:, p, :], R[:, p, :], ctmp)
                    else:
                        nc.vector.tensor_scalar_mul(
                            ctmp[:, sh:], y[:, :TC - sh], cw_s[:, p, k:k + 1]
                        )
                        nc.vector.tensor_add(
                            R[:, p, sh:], R[:, p, sh:], ctmp[:, sh:]
                        )

            def outproj_chunk(j, R):
                t0 = j * TC
                for mt in range(TC // 128):
                    ot = work.tile([128, C], f32, tag="ot", bufs=2)
                    for nb in range(2):
                        po = pps.tile([128, TC], f32, tag="ps")
                        for p in range(2):
                            nc.tensor.matmul(
                                po,
                                R[:, p, mt * 128:(mt + 1) * 128],
                                wo_s[:, p, nb * TC:(nb + 1) * TC],
                                start=(p == 0),
                                stop=(p == 1),
                            )
                        nc.any.tensor_copy(ot[:, nb * TC:(nb + 1) * TC], po)
                    nc.sync.dma_start(
                        out=out[t0 + mt * 128:t0 + (mt + 1) * 128, :], in_=ot
                    )

            # ---- program: interleaved chunk pipeline. proj(j+2) is issued
            # between attention chunks so its matmuls have lower priority
            # than attn(j) but higher than attn(j+1) — the scheduler then
            # uses them as PE filler during attn(j)'s exp waits, keeping
            # the HAM clock gate warm.
            def attn_chunk(j):
                # head order (1,0,3,2): the last head of each pair is par=0,
                # whose Y lands via DVE directly (no shift DMA) — shortens
                # the chain into conv_pair.
                R = work.tile([128, 2, TC], mdt, tag="R", bufs=2, name=f"R{j}")
                first = None
                for pos, hl in enumerate((1, 0, 3, 2)):
                    mm = attn_head(j, hl, gbs[j])
                    if first is None:
                        first = mm
                    if pos == 1:
                        conv_pair(j, 0, R)
                    elif pos == 3:
                        conv_pair(j, 1, R)
                outproj_chunk(j, R)
                return first

            gbs = [None] * NTC
            gbs[0] = proj_chunk(0)
            proj_mem()
            gbs[1] = proj_chunk(1)
            attn_chunk(0)
            gbs[2] = proj_chunk(2)
            a1 = attn_chunk(1)
            gbs[3] = proj_chunk(3)
            a2 = attn_chunk(2)
            attn_chunk(3)
            # ration PE filler: x loads for chunk j+1's projections only
            # become ready once attn(j) has started, so proj matmuls land
            # in the exp-wait bubbles of the later (starved) chunks.
            from concourse.tile import add_dep_helper
            for dma in xh_dmas[2]:
                add_dep_helper(dma.ins, a1.ins, reason="ration proj2 into attn1")
            for dma in xh_dmas[3]:
                add_dep_helper(dma.ins, a2.ins, reason="ration proj3 into attn2")
            if dbgY is not None:
                nc.sync.dma_start(
                    out=dbgY, in_=Y_s.rearrange("p a t -> p (a t)")
                )

    nc.compile()
    return nc


def _get_program():
    global _BUILT
    if _BUILT is None:
        _install_ntff_hook()
        _BUILT = _build_program()
    return _BUILT


# --------------------------------------------------------------- host side
def _tf32_round(a):
    """Cast to the matmul-operand dtype: TF32-round for float32r (data stays
    fp32 bits), bfloat16 for bf16 mode, passthrough for float32."""
    if _MM_DTYPE == "bfloat16":
        import ml_dtypes

        return np.ascontiguousarray(a, np.float32).astype(ml_dtypes.bfloat16)
    if _MM_DTYPE != "float32r":
        return np.ascontiguousarray(a, np.float32)
    u = np.ascontiguousarray(a, np.float32).view(np.uint32).astype(np.uint64)
    u = (u + 0x0FFF + ((u >> 13) & 1)) & np.uint64(0xFFFFE000)
    return u.astype(np.uint32).view(np.float32)


def host_prep(inputs):
    x = np.ascontiguousarray(np.asarray(inputs["x"], np.float32))
    fwd = np.asarray(inputs["fwd_mem"], np.float32)
    rev = np.asarray(inputs["rev_mem"], np.float32)
    Wq = np.asarray(inputs["Wq"], np.float32)
    Wk = np.asarray(inputs["Wk"], np.float32)
    Wv = np.asarray(inputs["Wv"], np.float32)
    Wo = np.asarray(inputs["Wo"], np.float32)
    gate_w = np.asarray(inputs["gate_w"], np.float32)
    gate_b = np.asarray(inputs["gate_b"], np.float32)
    canon_w = np.asarray(inputs["canon_w"], np.float32)
    canon_bias = np.asarray(inputs["canon_bias"], np.float32)

    Wg = (gate_w.astype(np.float64) @ Wq.astype(np.float64)).astype(np.float32)

    per_b, per_g = [], []
    for b in range(B):
        per_b.append({
            "xT": _tf32_round(x[b].T),
            "memT": _tf32_round(np.concatenate([fwd[b], rev[b]], axis=0).T),
        })
    for g in range(G):
        cs = slice(g * CPG, (g + 1) * CPG)
        WvTa = np.zeros((C, 65 * HPG), np.float32)
        for h in range(HPG):
            rows = Wv[g * CPG + h * HD: g * CPG + (h + 1) * HD]
            WvTa[:, 65 * h:65 * h + 64] = rows.T
        hs = slice(g * HPG, (g + 1) * HPG)
        per_g.append({
            "WqT": _tf32_round(Wq[cs].T),
            "WkT": _tf32_round(Wk[cs].T),
            "WvTa": _tf32_round(WvTa),
            "WgT": _tf32_round(Wg[hs].T),
            "gbn": np.ascontiguousarray(-gate_b[hs]).reshape(HPG, 1),
            "WoT": _tf32_round(Wo[:, cs].T),
            "cw": np.ascontiguousarray(canon_w[cs, 0, :]),
            "cb": np.ascontiguousarray(canon_bias[cs]).reshape(CPG, 1),
        })
    return per_b, per_g


LAST_EXEC_NS = None
LAST_RESULTS = None


def kernel(**inputs):
    global LAST_EXEC_NS, LAST_RESULTS
    from concourse.bass_utils import run_bass_kernel_spmd

    nc = _get_program()
    per_b, per_g = host_prep(inputs)
    in_maps = []
    for core in range(8):
        b, g = divmod(core, G)
        m = {}
        m.update(per_b[b])
        m.update(per_g[g])
        in_maps.append(m)

    trace = bool(int(os.environ.get("KERNEL_TRACE", "0")))
    kw = {}
    if trace:
        tcores = os.environ.get("KERNEL_TRACE_CORES", "0")
        kw = dict(
            trace=True,
            trace_cores=[int(c) for c in tcores.split(",")],
            tmpdir=os.environ.get("KERNEL_TRACE_DIR", None),
        )
    res = run_bass_kernel_spmd(nc, in_maps, core_ids=list(range(8)), **kw)
    LAST_EXEC_NS = res.exec_time_ns
    LAST_RESULTS = res
    outp = np.zeros((B, T, C), np.float32)
    for core in range(8):
        b = core // G
        outp[b] += res.results[core]["out"]
    return outp


# revision 27
# speedup vs baseline: 1.1082x; 1.1082x over previous
"""Trainium2 Bass kernel for nn_CMAModel (memory-augmented causal attention).

Sharding: 8 cores = 2 batches x 4 head-groups. Each core handles one batch and
4 heads (256 channels); the output projection is row-parallel and the 4
per-batch partials are summed on the host.

v2: chunk-major software pipeline so the tensor engine always has independent
matmul work queued (keeps the HAM clock gate at 2.4 GHz):
  projections for all 512-col T-chunks are issued first (separate PSUM pool
  tags: proj/outproj=2, scores=2, accum=4 banks), then per chunk: attention
  for the 4 heads, depthwise conv, out-projection. The Tile scheduler
  interleaves projection and out-projection matmuls into exp-wait bubbles.
Combine avoids the v1 DRAM round trips: 1/Z via reciprocal_approx_fast on the
PV ones-row, one partition-broadcast DMA per head-chunk, gate-row broadcasts
precomputed per chunk off the critical path.
"""
import contextlib
import ctypes
import os
import sys
import types

import numpy as np

# ---------------------------------------------------------------- constants
B, T, C = 2, 2048, 1024
H, HD = 16, 64
M = 256
G = 4                 # head-groups (cores per batch)
HPG = H // G          # 4 heads per core
CPG = HPG * HD        # 256 channels per core
S = T + 2 * M         # 2560 kv rows
SM = 2 * M            # 512 memory rows
NKT = C // 128        # 8 contraction tiles
NST = S // 128        # 20 S tiles (16 chunk + 4 mem)
TC = 512              # T chunk size
NTC = T // TC         # 4
K_CONV = 4
SCALE = 1.0 / float(np.sqrt(HD))

_MM_DTYPE = os.environ.get("BASS_MM_DTYPE", "float32r")
_ZBCAST_SBUF = bool(int(os.environ.get("BASS_ZBCAST_SBUF", "0")))
_RECIP_EXACT = bool(int(os.environ.get("BASS_RECIP_EXACT", "0")))

_BUILT = None


# ------------------------------------------------------- axon NTFF hook shim
def _install_ntff_hook():
    """The agent image lacks antenv.axon_hooks; synthesize it so
    run_bass_kernel_spmd(trace=True) can capture NTFF profiles."""
    if "antenv.axon_hooks" in sys.modules:
        return
    so_path = "/opt/axon/libaxon_pjrt.so"
    hook = None
    if os.path.exists(so_path):
        try:
            lib = ctypes.CDLL(so_path)
            if hasattr(lib, "axon_start_nrt_profile"):
                lib.axon_start_nrt_profile.argtypes = [
                    ctypes.POINTER(ctypes.c_int64),
                    ctypes.c_size_t,
                ]
                lib.axon_start_nrt_profile.restype = ctypes.c_int64
                lib.axon_stop_nrt_profile.argtypes = [ctypes.c_char_p]
                lib.axon_stop_nrt_profile.restype = ctypes.c_int64

                @contextlib.contextmanager
                def _hook(output_dir, device_ids):
                    import jax

                    jax.devices()
                    if device_ids:
                        ids = (ctypes.c_int64 * len(device_ids))(*device_ids)
                        rc = lib.axon_start_nrt_profile(ids, len(device_ids))
                    else:
                        rc = lib.axon_start_nrt_profile(None, 0)
                    if rc != 0:
                        raise RuntimeError(f"axon_start_nrt_profile rc={rc}")
                    try:
                        yield
                    finally:
                        n = lib.axon_stop_nrt_profile(str(output_dir).encode())
                        if n < 0:
                            raise RuntimeError(f"axon_stop_nrt_profile rc={n}")

                hook = _hook
        except OSError:
            pass
    mod = types.ModuleType("antenv.axon_hooks")
    mod.get_axon_ntff_profile_hook = lambda: hook
    mod.set_axon_ntff_profile_hook = lambda h: None
    sys.modules["antenv.axon_hooks"] = mod


# ------------------------------------------------------------- device build
def _build_program():
    import concourse.tile as tile
    from concourse import bacc, mybir
    from concourse.masks import make_upper_triangular

    f32 = mybir.dt.float32
    bf16 = mybir.dt.bfloat16
    mdt = getattr(mybir.dt, _MM_DTYPE)  # dtype of all matmul operands

    nc = bacc.Bacc("TRN2", target_bir_lowering=False, debug=False, num_devices=8)

    xT = nc.dram_tensor("xT", [C, T], mdt, kind="ExternalInput").ap()
    memT = nc.dram_tensor("memT", [C, SM], mdt, kind="ExternalInput").ap()
    WqT = nc.dram_tensor("WqT", [C, CPG], mdt, kind="ExternalInput").ap()
    WkT = nc.dram_tensor("WkT", [C, CPG], mdt, kind="ExternalInput").ap()
    WvTa = nc.dram_tensor("WvTa", [C, 65 * HPG], mdt, kind="ExternalInput").ap()
    WgT = nc.dram_tensor("WgT", [C, HPG], mdt, kind="ExternalInput").ap()
    gbn = nc.dram_tensor("gbn", [HPG, 1], f32, kind="ExternalInput").ap()
    WoT = nc.dram_tensor("WoT", [CPG, C], mdt, kind="ExternalInput").ap()
    cw = nc.dram_tensor("cw", [CPG, K_CONV], f32, kind="ExternalInput").ap()
    cb = nc.dram_tensor("cb", [CPG, 1], f32, kind="ExternalInput").ap()
    out = nc.dram_tensor("out", [T, C], f32, kind="ExternalOutput").ap()
    dbg_on = bool(int(os.environ.get("BASS_DBG", "0")))
    dbgZ = (
        nc.dram_tensor("dbgZ", [NTC * HPG, TC], f32, kind="ExternalOutput").ap()
        if dbg_on else None
    )
    dbgY = (
        nc.dram_tensor("dbgY", [128, 2 * T], mybir.dt.bfloat16,
                       kind="ExternalOutput").ap()
        if dbg_on else None
    )

    Exp = mybir.ActivationFunctionType.Exp

    with tile.TileContext(nc) as tc:
        with contextlib.ExitStack() as ctx:
            const = ctx.enter_context(tc.tile_pool(name="const", bufs=1))
            xpool = ctx.enter_context(tc.tile_pool(name="xpool", bufs=2))
            sb = ctx.enter_context(tc.tile_pool(name="sb", bufs=1))
            work = ctx.enter_context(tc.tile_pool(name="work", bufs=3))
            small = ctx.enter_context(tc.tile_pool(name="small", bufs=1))
            pps = ctx.enter_context(tc.tile_pool(name="pps", bufs=1, space="PSUM"))
            psc = ctx.enter_context(tc.tile_pool(name="psc", bufs=2, space="PSUM"))
            ppa = ctx.enter_context(tc.tile_pool(name="ppa", bufs=3, space="PSUM"))
            drs = ctx.enter_context(tc.tile_pool(name="drs", bufs=4, space="DRAM"))

            # ---- constants / weights (q/k first so chunk-0 proj starts early)
            wq_s = const.tile([128, NKT, CPG], mdt)
            nc.sync.dma_start(out=wq_s, in_=WqT.rearrange("(a p) n -> p a n", p=128))
            wk_s = const.tile([128, NKT, CPG], mdt)
            nc.sync.dma_start(out=wk_s, in_=WkT.rearrange("(a p) n -> p a n", p=128))
            wva_s = const.tile([128, NKT, 65 * HPG], mdt)
            nc.sync.dma_start(out=wva_s, in_=WvTa.rearrange("(a p) n -> p a n", p=128))
            wg_s = const.tile([128, NKT, HPG], mdt)
            nc.sync.dma_start(out=wg_s, in_=WgT.rearrange("(a p) n -> p a n", p=128))
            cw_s = const.tile([128, 2, K_CONV], f32)
            nc.sync.dma_start(out=cw_s, in_=cw.rearrange("(a p) n -> p a n", p=128))
            cb_s = const.tile([128, 2, 1], f32)
            nc.sync.dma_start(out=cb_s, in_=cb.rearrange("(a p) n -> p a n", p=128))
            gbn_s = const.tile([HPG, 1], f32)
            nc.sync.dma_start(out=gbn_s, in_=gbn)
            wo_s = const.tile([128, 2, C], mdt)
            nc.sync.dma_start(out=wo_s, in_=WoT.rearrange("(a p) n -> p a n", p=128))

            tri = const.tile([128, 128], f32)
            make_upper_triangular(nc, tri, val=1.0, diag=True)

            # ---- persistent activations
            qT_s = sb.tile([128, 2, T], mdt)
            kT_s = sb.tile([128, 2, S], mdt)
            V_s = sb.tile([128, NST, 65 * HPG], mdt)  # [128, 20, 260]
            Y_s = sb.tile([128, 2, T], bf16)          # combined attention output

            gd = drs.tile([NTC, HPG, TC], bf16, tag="gd", bufs=1)  # gate rows

            xTr = xT.rearrange("(a p) t -> p a t", p=128)

            def ones_fill(st):
                oc = V_s[:, st, 64:65 * HPG:65]
                nc.vector.tensor_scalar(
                    oc, oc, 0.0, 1.0,
                    mybir.AluOpType.mult, mybir.AluOpType.add,
                )

            xh_dmas = {}  # j -> list of x-load DMA instructions (for rationing)

            def proj_chunk(j):
                """q/k/V/gate projections for T columns [j*TC, (j+1)*TC)."""
                t0 = j * TC
                xh = xpool.tile([128, NKT, TC], mdt, tag="xc", name=f"xh{j}")
                xh_dmas[j] = []
                for k in range(NKT):
                    xh_dmas[j].append(
                        nc.sync.dma_start(out=xh[:, k, :], in_=xTr[:, k, t0:t0 + TC])
                    )
                for m in range(2):
                    pq = pps.tile([128, TC], f32, tag="ps")
                    for k in range(NKT):
                        nc.tensor.matmul(
                            pq,
                            wq_s[:, k, m * 128:(m + 1) * 128],
                            xh[:, k, :],
                            start=(k == 0),
                            stop=(k == NKT - 1),
                        )
                    nc.vector.tensor_copy(qT_s[:, m, t0:t0 + TC], pq)
                    pk = pps.tile([128, TC], f32, tag="ps")
                    for k in range(NKT):
                        nc.tensor.matmul(
                            pk,
                            wk_s[:, k, m * 128:(m + 1) * 128],
                            xh[:, k, :],
                            start=(k == 0),
                            stop=(k == NKT - 1),
                        )
                    nc.vector.tensor_copy(kT_s[:, m, t0:t0 + TC], pk)
                for mt in range(TC // 128):
                    st = j * 4 + mt
                    pv = pps.tile([128, 65 * HPG], f32, tag="ps")
                    for k in range(NKT):
                        nc.tensor.matmul(
                            pv,
                            xh[:, k, mt * 128:(mt + 1) * 128],
                            wva_s[:, k, :],
                            start=(k == 0),
                            stop=(k == NKT - 1),
                        )
                    nc.vector.tensor_copy(V_s[:, st, :], pv)
                    ones_fill(st)
                # gate: logits -> exp(-(l+b)) -> +1 -> 1/x -> DRAM -> bcast
                pg = pps.tile([HPG, TC], f32, tag="ps")
                for k in range(NKT):
                    nc.tensor.matmul(
                        pg,
                        wg_s[:, k, :],
                        xh[:, k, :],
                        start=(k == 0),
                        stop=(k == NKT - 1),
                    )
                gtmp = small.tile([HPG, TC], f32, tag="gtmp", bufs=2)
                nc.scalar.activation(gtmp, pg, Exp, bias=gbn_s, scale=-1.0)
                gr = small.tile([128, HPG, TC // 128], f32, tag="gr", bufs=2)
                for hl in range(HPG):
                    nc.sync.dma_start(out=gr[:, hl, :], in_=gtmp[hl:hl + 1, :])
                nc.vector.tensor_scalar_add(gr, gr, 1.0)
                nc.vector.reciprocal(gr, gr)
                grb = small.tile([128, HPG, TC // 128], bf16, tag="grb", bufs=2)
                nc.vector.tensor_copy(grb, gr)
                for hl in range(HPG):
                    nc.sync.dma_start(out=gd[j, hl:hl + 1, :], in_=grb[:, hl, :])
                gb = small.tile([64, HPG, TC], bf16, tag="gb", bufs=2,
                                name=f"gb{j}")
                for hl in range(HPG):
                    nc.gpsimd.dma_start(
                        out=gb[:, hl, :],
                        in_=gd[j, hl:hl + 1, :].partition_broadcast(64),
                    )
                return gb

            def proj_mem():
                mems = xpool.tile([128, NKT, SM], mdt, tag="xc", name="mems")
                nc.sync.dma_start(
                    out=mems, in_=memT.rearrange("(a p) t -> p a t", p=128)
                )
                for m in range(2):
                    pk = pps.tile([128, SM], f32, tag="ps")
                    for half in range(2):
                        for k in range(NKT):
                            nc.tensor.matmul(
                                pk[:, half * 256:(half + 1) * 256],
                                wk_s[:, k, m * 128:(m + 1) * 128],
                                mems[:, k, half * 256:(half + 1) * 256],
                                start=(k == 0),
                                stop=(k == NKT - 1),
                            )
                    nc.vector.tensor_copy(kT_s[:, m, T:], pk)
                for mt in range(SM // 128):
                    st = 16 + mt
                    pv = pps.tile([128, 65 * HPG], f32, tag="ps")
                    for k in range(NKT):
                        nc.tensor.matmul(
                            pv,
                            mems[:, k, mt * 128:(mt + 1) * 128],
                            wva_s[:, k, :],
                            start=(k == 0),
                            stop=(k == NKT - 1),
                        )
                    nc.vector.tensor_copy(V_s[:, st, :], pv)
                    ones_fill(st)

            def attn_head(j, hl, gb):
                """scores+exp+mask+PV+combine for head hl on chunk j."""
                mq, par = divmod(hl, 2)
                ro = 64 * par
                vc = 65 * hl
                nct = 4 * (j + 1)
                Ac = ppa.tile([128, TC], f32, tag="pa")
                Am = ppa.tile([128, TC], f32, tag="pa")
                first_mm = None
                for ip in range((nct + 4) // 2):
                    sc2 = psc.tile([128, 2, TC], f32, tag="sc")
                    Pt2 = work.tile([128, 2, TC], mdt, tag="P")
                    halves = []
                    for half in range(2):
                        i = 2 * ip + half
                        is_mem = i >= nct
                        si = (16 + i - nct) if is_mem else i
                        off = 0
                        if not is_mem and si >= 4 * j:
                            off = 128 * si - TC * j
                        halves.append((i, is_mem, si, off))
                        mm = nc.tensor.matmul(
                            sc2[:, half, off:],
                            kT_s[ro:ro + 64, mq, si * 128:(si + 1) * 128],
                            qT_s[ro:ro + 64, mq, TC * j + off:TC * (j + 1)],
                            start=True,
                            stop=True,
                        )
                        if first_mm is None:
                            first_mm = mm
                    if halves[1][3] > 0:
                        # diagonal pair: separate trimmed exps (avoid
                        # reading unwritten PSUM between the valid ranges)
                        for half, (i, is_mem, si, off) in enumerate(halves):
                            nc.scalar.activation(
                                Pt2[:, half, off:], sc2[:, half, off:],
                                Exp, scale=SCALE,
                            )
                    else:
                        off0 = halves[0][3]
                        nc.scalar.activation(
                            Pt2[:, :, :].rearrange("p a t -> p (a t)")[:, off0:],
                            sc2[:, :, :].rearrange("p a t -> p (a t)")[:, off0:],
                            Exp, scale=SCALE,
                        )
                    for half, (i, is_mem, si, off) in enumerate(halves):
                        if not is_mem and si >= 4 * j:
                            nc.vector.tensor_mul(
                                Pt2[:, half, off:off + 128],
                                Pt2[:, half, off:off + 128], tri
                            )
                        dst = Am if is_mem else Ac
                        first = (i == 0) or (is_mem and i == nct)
                        last = (i == nct - 1) or (i == nct + 3)
                        nc.tensor.matmul(
                            dst[0:65, off:],
                            V_s[:, si, vc:vc + 65],
                            Pt2[:, half, off:],
                            start=first,
                            stop=last,
                        )
                # ---- combine: Y = (Ac + g*Am) / Z
                zrow = small.tile([128, TC], f32, tag="zrow", bufs=3)
                zr = zrow[64:65, :]
                nc.vector.tensor_copy(zr, Ac[64:65, :])
                nc.vector.tensor_add(zr, zr, Am[64:65, :])
                zq = small.tile([128, TC // 128], f32, tag="zq", bufs=3)
                nc.sync.dma_start(out=zq, in_=zr)
                nc.vector.reciprocal(zq, zq)
                zb = small.tile([64, TC], f32, tag="zb", bufs=3)
                if dbgZ is not None:
                    idx = j * HPG + hl
                    nc.sync.dma_start(out=dbgZ[idx:idx + 1, :], in_=zq)
                    nc.sync.dma_start(
                        out=zb, in_=dbgZ[idx:idx + 1, :].partition_broadcast(64)
                    )
                else:
                    zd = drs.tile([1, TC], f32, tag="zd", bufs=4)
                    nc.sync.dma_start(out=zd, in_=zq)
                    nc.sync.dma_start(out=zb, in_=zd.partition_broadcast(64))
                t1 = small.tile([64, TC], f32, tag="t1", bufs=2)
                nc.vector.tensor_mul(t1, gb[:, hl, :], Am[0:64, :])
                nc.vector.tensor_add(t1, t1, Ac[0:64, :])
                if par == 0:
                    nc.vector.tensor_mul(
                        Y_s[0:64, mq, TC * j:TC * (j + 1)], t1, zb
                    )
                else:
                    yt = small.tile([64, TC], bf16, tag="yt", bufs=3)
                    nc.vector.tensor_mul(yt, t1, zb)
                    nc.gpsimd.dma_start(
                        out=Y_s[64:128, mq, TC * j:TC * (j + 1)], in_=yt
                    )
                return first_mm

            def conv_pair(j, p, R):
                """depthwise causal conv + residual + bias for chunk j, bank p."""
                t0 = j * TC
                y = Y_s[:, p, t0:t0 + TC]
                nc.vector.tensor_scalar_add(R[:, p, :], y, cb_s[:, p, :])
                for k in range(K_CONV):
                    sh = K_CONV - 1 - k
                    ctmp = small.tile([128, TC], f32, tag="ctmp", bufs=2)
                    if sh == 0:
                        nc.vector.tensor_scalar_mul(ctmp, y, cw_s[:, p, k:k + 1])
                        nc.vector.tensor_add(R[:, p, :], R[:, p, :], ctmp)
                    elif t0 - sh >= 0:
                        nc.vector.tensor_scalar_mul(
                            ctmp, Y_s[:, p, t0 - sh:t0 + TC - sh],
                            cw_s[:, p, k:k + 1],
                        )
                        nc.vector.tensor_add(R[:, p, :], R[:, p, :], ctmp)
                    else:
                        nc.vector.tensor_scalar_mul(
                            ctmp[:, sh:], y[:, :TC - sh], cw_s[:, p, k:k + 1]
                        )
                        nc.vector.tensor_add(
                            R[:, p, sh:], R[:, p, sh:], ctmp[:, sh:]
                        )

            def outproj_chunk(j, R):
                t0 = j * TC
                for mt in range(TC // 128):
                    ot = work.tile([128, C], f32, tag="ot", bufs=2)
                    for nb in range(2):
                        po = pps.tile([128, TC], f32, tag="ps")
                        for p in range(2):
                            nc.tensor.matmul(
                                po,
                                R[:, p, mt * 128:(mt + 1) * 128],
                                wo_s[:, p, nb * TC:(nb + 1) * TC],
                                start=(p == 0),
                                stop=(p == 1),
                            )
                        nc.any.tensor_copy(ot[:, nb * TC:(nb + 1) * TC], po)
                    nc.sync.dma_start(
                        out=out[t0 + mt * 128:t0 + (mt + 1) * 128, :], in_=ot
                    )

            # ---- program: interleaved chunk pipeline. proj(j+2) is issued
            # between attention chunks so its matmuls have lower priority
            # than attn(j) but higher than attn(j+1) — the scheduler then
            # uses them as PE filler during attn(j)'s exp waits, keeping
            # the HAM clock gate warm.
            def attn_chunk(j):
                # head order (1,0,3,2): the last head of each pair is par=0,
                # whose Y lands via DVE directly (no shift DMA) — shortens
                # the chain into conv_pair.
                R = work.tile([128, 2, TC], mdt, tag="R", bufs=2, name=f"R{j}")
                first = None
                for pos, hl in enumerate((1, 0, 3, 2)):
                    mm = attn_head(j, hl, gbs[j])
                    if first is None:
                        first = mm
                    if pos == 1:
                        conv_pair(j, 0, R)
                    elif pos == 3:
                        conv_pair(j, 1, R)
                outproj_chunk(j, R)
                return first

            gbs = [None] * NTC
            gbs[0] = proj_chunk(0)
            proj_mem()
            gbs[1] = proj_chunk(1)
            attn_chunk(0)
            gbs[2] = proj_chunk(2)
            attn_chunk(1)
            gbs[3] = proj_chunk(3)
            attn_chunk(2)
            attn_chunk(3)
            if dbgY is not None:
                nc.sync.dma_start(
                    out=dbgY, in_=Y_s.rearrange("p a t -> p (a t)")
                )

    nc.compile()
    return nc


def _get_program():
    global _BUILT
    if _BUILT is None:
        _install_ntff_hook()
        _BUILT = _build_program()
    return _BUILT


# --------------------------------------------------------------- host side
def _tf32_round(a):
    """Cast to the matmul-operand dtype: TF32-round for float32r (data stays
    fp32 bits), bfloat16 for bf16 mode, passthrough for float32."""
    if _MM_DTYPE == "bfloat16":
        import ml_dtypes

        return np.ascontiguousarray(a, np.float32).astype(ml_dtypes.bfloat16)
    if _MM_DTYPE != "float32r":
        return np.ascontiguousarray(a, np.float32)
    u = np.ascontiguousarray(a, np.float32).view(np.uint32).astype(np.uint64)
    u = (u + 0x0FFF + ((u >> 13) & 1)) & np.uint64(0xFFFFE000)
    return u.astype(np.uint32).view(np.float32)


def host_prep(inputs):
    x = np.ascontiguousarray(np.asarray(inputs["x"], np.float32))
    fwd = np.asarray(inputs["fwd_mem"], np.float32)
    rev = np.asarray(inputs["rev_mem"], np.float32)
    Wq = np.asarray(inputs["Wq"], np.float32)
    Wk = np.asarray(inputs["Wk"], np.float32)
    Wv = np.asarray(inputs["Wv"], np.float32)
    Wo = np.asarray(inputs["Wo"], np.float32)
    gate_w = np.asarray(inputs["gate_w"], np.float32)
    gate_b = np.asarray(inputs["gate_b"], np.float32)
    canon_w = np.asarray(inputs["canon_w"], np.float32)
    canon_bias = np.asarray(inputs["canon_bias"], np.float32)

    Wg = (gate_w.astype(np.float64) @ Wq.astype(np.float64)).astype(np.float32)

    per_b, per_g = [], []
    for b in range(B):
        per_b.append({
            "xT": _tf32_round(x[b].T),
            "memT": _tf32_round(np.concatenate([fwd[b], rev[b]], axis=0).T),
        })
    for g in range(G):
        cs = slice(g * CPG, (g + 1) * CPG)
        WvTa = np.zeros((C, 65 * HPG), np.float32)
        for h in range(HPG):
            rows = Wv[g * CPG + h * HD: g * CPG + (h + 1) * HD]
            WvTa[:, 65 * h:65 * h + 64] = rows.T
        hs = slice(g * HPG, (g + 1) * HPG)
        per_g.append({
            "WqT": _tf32_round(Wq[cs].T),
            "WkT": _tf32_round(Wk[cs].T),
            "WvTa": _tf32_round(WvTa),
            "WgT": _tf32_round(Wg[hs].T),
            "gbn": np.ascontiguousarray(-gate_b[hs]).reshape(HPG, 1),
            "WoT": _tf32_round(Wo[:, cs].T),
            "cw": np.ascontiguousarray(canon_w[cs, 0, :]),
            "cb": np.ascontiguousarray(canon_bias[cs]).reshape(CPG, 1),
        })
    return per_b, per_g


LAST_EXEC_NS = None
LAST_RESULTS = None


def kernel(**inputs):
    global LAST_EXEC_NS, LAST_RESULTS
    from concourse.bass_utils import run_bass_kernel_spmd

    nc = _get_program()
    per_b, per_g = host_prep(inputs)
    in_maps = []
    for core in range(8):
        b, g = divmod(core, G)
        m = {}
        m.update(per_b[b])
        m.update(per_g[g])
        in_maps.append(m)

    trace = bool(int(os.environ.get("KERNEL_TRACE", "0")))
    kw = {}
    if trace:
        tcores = os.environ.get("KERNEL_TRACE_CORES", "0")
        kw = dict(
            trace=True,
            trace_cores=[int(c) for c in tcores.split(",")],
            tmpdir=os.environ.get("KERNEL_TRACE_DIR", None),
        )
    res = run_bass_kernel_spmd(nc, in_maps, core_ids=list(range(8)), **kw)
    LAST_EXEC_NS = res.exec_time_ns
    LAST_RESULTS = res
    outp = np.zeros((B, T, C), np.float32)
    for core in range(8):
        b = core // G
        outp[b] += res.results[core]["out"]
    return outp
